# revision 30
# baseline (speedup 1.0000x reference)
"""L2-distance attention (degree-0 DTP block) on 8 Trainium2 NeuronCores.

Sharding: 512 (b,n) nodes split 64 per core -> 1024 edges per core; cores
0-3 serve batch 0, cores 4-7 batch 1, so each core only needs its batch's
256 node features. Layout: channel-major (feature channels on SBUF
partitions, edges on the free dim). Neighbor/center gathers are one-hot
selector matmuls; the selectors are built ON DEVICE from the uploaded
index rows (partition-broadcast matmul + is_equal against an iota column),
so the per-call host->device traffic is just features + indices + mask +
distances (~0.6 MB). All weight-derived operands are device-resident and
cached across calls keyed on a content hash; the jit executable is built
once. The per-edge radial contraction kv[o,e] = sum_{r,d} W3[r,o,d]*
hdd[r,e]*xe[d,e] runs as a bf16 GEMM against the Khatri-Rao factor
xs[(r,d),e] accumulated over 32 K-chunks in PSUM.
"""
import hashlib
import numpy as np
import ml_dtypes

import jax
from jax.sharding import Mesh, PartitionSpec, NamedSharding
from jax.experimental.shard_map import shard_map

import concourse.bacc as bacc
import concourse.bass as bass
import concourse.tile as tile
from concourse import mybir, bass2jax

F32 = mybir.dt.float32
BF16 = mybir.dt.bfloat16
AF = mybir.ActivationFunctionType
ALU = mybir.AluOpType

B, N, K, D = 2, 256, 16, 64
H, HID = 4, 128
KVD = 2 * HID
RH = 64
NCORES = 8
NODES = B * N                 # 512
PCORE = NODES // NCORES       # 64 nodes/core
E = PCORE * K                 # 1024 edges/core
NB = N                        # per-batch node space a core needs (256)
NCH = NB // 128               # 2 selector chunks
SCALE = (HID // H) ** -0.5


def _emit(nc, tc, P, out, ctx):
    cst = ctx.enter_context(tc.tile_pool(name="cst", bufs=1))
    wk = ctx.enter_context(tc.tile_pool(name="wk", bufs=1))
    lp = ctx.enter_context(tc.tile_pool(name="lp", bufs=3))
    ps = ctx.enter_context(tc.tile_pool(name="ps", bufs=1, space="PSUM"))
    dr = ctx.enter_context(tc.tile_pool(name="dr", bufs=1, space="DRAM"))

    def load(name, dt=F32):
        t = cst.tile(list(P[name].shape), dt, tag=name)
        nc.sync.dma_start(out=t[...], in_=P[name].ap())
        return t

    # DMA order = need order: tiny early-phase tensors first so the selector
    # build and prenorm aren't stuck behind the 2MB W3sb stream.
    iota2 = load("iota2")
    fT = load("fT"); nsc = load("nsc"); mm64 = load("mm64", BF16)
    rdT = load("rdT", BF16)
    W1 = load("W1", BF16); b1 = load("b1"); g1 = load("g1")
    W2 = load("W2", BF16); b2 = load("b2"); g2 = load("g2")
    Wq = load("Wq", BF16); Wxi = load("Wxi", BF16)
    WxjI = load("WxjI", BF16)
    W3t = load("W3sb", BF16); b3T = load("b3T", BF16)
    Wkv = load("Wkv", BF16); Wout = load("Wout")
    hredb = load("hredb", BF16); hexpb = load("hexpb", BF16)

    eps64 = cst.tile([RH, 1], F32); nc.vector.memset(eps64[...], 1e-5)

    def pt(tag, p=128, w=512):
        return ps.tile([p, w], F32, tag=tag, name=tag)

    def dbc(param, parts, dt=BF16):
        """DMA-broadcast a [1, E] DRAM input across `parts` partitions."""
        t = cst.tile([parts, E], dt, tag="bc_" + param)
        src = bass.AP(tensor=P[param], offset=0, ap=[[0, parts], [1, E]])
        nc.sync.dma_start(out=t[...], in_=src)
        return t

    # ---------------- selectors built on device from index rows -------------
    # Sg[p, ch, e] = 1 iff neighbor(e) == p + 128*ch   (bf16 one-hot);
    # the index row is partition-broadcast by DMA (stride-0 source AP), so
    # no TensorE work is spent here.
    gnB = dbc("gnb", 128)
    gcB = dbc("gcb", 128)
    Sg = wk.tile([128, NCH, E], BF16)
    Cg = wk.tile([128, NCH, E], BF16)
    for src, dstT in ((gnB, Sg), (gcB, Cg)):
        for ch in range(NCH):
            nc.vector.tensor_scalar(dstT[:, ch, :], src[...],
                                    iota2[:, ch:ch + 1], None, ALU.is_equal)

    # -------- prenorm: xT = fT * rsqrt(max(mean f^2, eps)) * norm_scale -----
    # stats broadcast to all D partitions via the (1/64)-ones matmul, so
    # every elementwise op below is partition-parallel (no [1,N] tensors)
    sqf = wk.tile([D, NB], BF16)
    nc.scalar.activation(out=sqf[...], in_=fT[...], func=AF.Square)
    SB = pt("pe", D)
    nc.tensor.matmul(SB[:D, :NB], mm64[:D, :D], sqf[...], start=True, stop=True)
    mx = wk.tile([D, NB], F32)
    nc.vector.tensor_scalar_max(out=mx[...], in0=SB[:D, :NB], scalar1=1e-24)
    sdB = wk.tile([D, NB], F32)
    nc.scalar.activation(out=sdB[...], in_=mx[...], func=AF.Sqrt)
    rinvB = wk.tile([D, NB], F32)
    nc.vector.reciprocal_approx_fast(out=rinvB[...], in_=sdB[...])
    xt1 = wk.tile([D, NB], F32)
    nc.vector.tensor_tensor(out=xt1[...], in0=fT[...], in1=rinvB[...], op=ALU.mult)
    xT = wk.tile([D, NB], BF16)
    nc.vector.tensor_scalar_mul(out=xT[...], in0=xt1[...], scalar1=nsc[...])

    # ---------- node-major chunks: [x@Wxj | x] via one matmul per chunk ------
    x_nm, xj_nm = [], []
    for ch in range(NCH):
        pp = pt("pe")
        nc.tensor.matmul(pp[:, :2 * D], xT[:, ch * 128:(ch + 1) * 128],
                         WxjI[...], start=True, stop=True)
        xj = wk.tile([128, D], BF16, tag=f"xj{ch}", name=f"xj{ch}")
        nc.scalar.copy(out=xj[...], in_=pp[:, :D])
        xn = wk.tile([128, D], BF16, tag=f"xn{ch}", name=f"xn{ch}")
        nc.scalar.copy(out=xn[...], in_=pp[:, D:2 * D])
        xj_nm.append(xj); x_nm.append(xn)

    # ---------- center replicate: xTe[d, e] = x[ctr(e), d] ----------
    xTe = wk.tile([D, E], BF16)
    for nch in range(2):
        pp = pt("pe" if nch == 0 else "pf", D)
        for ch in range(NCH):
            nc.tensor.matmul(pp[:D, :], x_nm[ch][...],
                             Cg[:, ch, nch * 512:(nch + 1) * 512],
                             start=(ch == 0), stop=(ch == NCH - 1))
        nc.scalar.copy(out=xTe[:, nch * 512:(nch + 1) * 512], in_=pp[:D, :])

    # ---------- edge features: xeT = xg(neighbor) + xi(center) ----------
    xeT_ps = []
    for nch in range(2):
        pp = pt("pa" if nch == 0 else "pb", D)
        xeT_ps.append(pp)
        for ch in range(NCH):
            nc.tensor.matmul(pp[:D, :], xj_nm[ch][...],
                             Sg[:, ch, nch * 512:(nch + 1) * 512],
                             start=(ch == 0), stop=False)
        nc.tensor.matmul(pp[:D, :], Wxi[...],
                         xTe[:, nch * 512:(nch + 1) * 512],
                         start=False, stop=True)
    stack = wk.tile([128, E], BF16)   # [xeT; xeT] bf16
    for nch in range(2):
        sl = slice(nch * 512, (nch + 1) * 512)
        nc.vector.tensor_copy(out=stack[:D, sl], in_=xeT_ps[nch][:D, :])
        nc.scalar.copy(out=stack[D:, sl], in_=xeT_ps[nch][:D, :])

    # ---------- queries per edge ----------
    qTe = wk.tile([HID, E], F32)
    for nch in range(2):
        pp = pt("pc")
        nc.tensor.matmul(pp[...], Wq[...], xTe[:, nch * 512:(nch + 1) * 512],
                         start=True, stop=True)
        nc.scalar.copy(out=qTe[:, nch * 512:(nch + 1) * 512], in_=pp[...])

    # ---------- radial MLP: 2 x (linear -> silu -> LN*g), channel-major ------
    # LN stats are broadcast to all RH partitions directly by the (1/64)-ones
    # matmul (muB = mean, q2B = E[z^2]), so the whole pipeline stays
    # partition-parallel with no [1,E] intermediates.
    def radial_layer(z_src_ps, bias, g, out_dt, tg):
        z = wk.tile([RH, E], BF16, tag=tg + "z", name=tg + "z")
        for nch in range(2):
            nc.scalar.activation(out=z[:, nch * 512:(nch + 1) * 512],
                                 in_=z_src_ps[nch][:RH, :], func=AF.Silu,
                                 bias=bias[...], scale=1.0)
        sq = wk.tile([RH, E], BF16, tag=tg + "q", name=tg + "q")
        nc.scalar.activation(out=sq[...], in_=z[...], func=AF.Square)
        hddo = wk.tile([RH, E], out_dt, tag=tg)
        for nch in range(2):
            sl = slice(nch * 512, (nch + 1) * 512)
            muB = pt("pc", RH)
            nc.tensor.matmul(muB[:RH, :], mm64[...], z[:, sl], start=True, stop=True)
            q2B = pt("pd", RH)
            nc.tensor.matmul(q2B[:RH, :], mm64[...], sq[:, sl], start=True, stop=True)
            m2 = wk.tile([RH, 512], F32, tag=tg + "m2", name=tg + "m2")
            nc.scalar.activation(out=m2[...], in_=muB[:RH, :], func=AF.Square)
            var = wk.tile([RH, 512], F32, tag=tg + "v", name=tg + "v")
            nc.vector.tensor_tensor(out=var[...], in0=q2B[:RH, :], in1=m2[...], op=ALU.subtract)
            sd = wk.tile([RH, 512], F32, tag=tg + "sd", name=tg + "sd")
            nc.scalar.activation(out=sd[...], in_=var[...], func=AF.Sqrt,
                                 bias=eps64[...], scale=1.0)  # sqrt(var+eps)
            rstd = wk.tile([RH, 512], F32, tag=tg + "rs", name=tg + "rs")
            nc.vector.reciprocal_approx_fast(out=rstd[...], in_=sd[...])
            d1 = wk.tile([RH, 512], F32, tag=tg + "d1", name=tg + "d1")
            nc.vector.tensor_tensor(out=d1[...], in0=z[:, sl], in1=muB[:RH, :], op=ALU.subtract)
            d2 = wk.tile([RH, 512], F32, tag=tg + "d2", name=tg + "d2")
            nc.vector.tensor_tensor(out=d2[...], in0=d1[...], in1=rstd[...], op=ALU.mult)
            nc.vector.tensor_scalar_mul(out=hddo[:, sl], in0=d2[...], scalar1=g[...])
        return hddo

    h1ps = []
    for nch in range(2):
        pp = pt("pe" if nch == 0 else "pf", RH)
        nc.tensor.matmul(pp[:RH, :], W1[...], rdT[:, nch * 512:(nch + 1) * 512],
                         start=True, stop=True)
        h1ps.append(pp)
    hdd1 = radial_layer(h1ps, b1, g1, BF16, "h1")
    h2ps = []
    for nch in range(2):
        pp = pt("pe" if nch == 0 else "pf", RH)
        nc.tensor.matmul(pp[:RH, :], W2[...], hdd1[:, nch * 512:(nch + 1) * 512],
                         start=True, stop=True)
        h2ps.append(pp)
    hddT = radial_layer(h2ps, b2, g2, BF16, "h2")

    # ---------- big GEMM: kv[o,e] = sum_{rd} W3'[rd,o] * xs[rd,e] ----------
    # The per-chunk hdd-row replication runs on the DMA engines (hddT is
    # bounced through DRAM once, then each chunk's two rows are read back
    # with a stride-0 partition AP), so TensorE spends rows only on the
    # 4 accumulation matmuls per chunk. DMA -> DVE -> PE pipeline is 3-deep
    # via the lp pool rotation.
    hdram = dr.tile([RH, E], BF16)
    nc.sync.dma_start(out=hdram[...], in_=hddT[...])
    kvtags = ["pa", "pb", "pc", "pd"]
    kvps = [[pt(kvtags[2 * m + n]) for n in range(2)] for m in range(2)]
    for c in range(32):
        hBs = lp.tile([128, E], BF16, tag="hBs", name="hBs")
        for half in range(2):
            r = 2 * c + half
            nc.sync.dma_start(out=hBs[half * RH:(half + 1) * RH, :],
                              in_=hdram[r:r + 1, :].broadcast_to([RH, E]))
        xs = lp.tile([128, E], BF16, tag="xs", name="xs")
        nc.vector.tensor_tensor(out=xs[...], in0=stack[...], in1=hBs[...],
                                op=ALU.mult)
        for m in range(2):
            for nch in range(2):
                nc.tensor.matmul(kvps[m][nch][...],
                                 W3t[:, c, m * 128:(m + 1) * 128],
                                 xs[:, nch * 512:(nch + 1) * 512],
                                 start=(c == 0), stop=False)
    for m in range(2):
        for nch in range(2):
            nc.tensor.matmul(kvps[m][nch][...], b3T[:, m * 128:(m + 1) * 128],
                             stack[:D, nch * 512:(nch + 1) * 512],
                             start=False, stop=True)
    kvT = wk.tile([128, 2, E], BF16)
    for m in range(2):
        for nch in range(2):
            nc.scalar.copy(out=kvT[:, m, nch * 512:(nch + 1) * 512],
                           in_=kvps[m][nch][...])

    # ---------- kv2 = Wkv^T @ kv : kk rows 0:128, vv rows 128:256 ----------
    kkT = wk.tile([HID, E], F32)
    vvT = wk.tile([HID, E], F32)
    for m, dst_t in ((0, kkT), (1, vvT)):
        for nch in range(2):
            pp = pt("pa" if nch == 0 else "pb")
            for kc in range(2):
                nc.tensor.matmul(pp[...],
                                 Wkv[:, kc, m * 128:(m + 1) * 128],
                                 kvT[:, kc, nch * 512:(nch + 1) * 512],
                                 start=(kc == 0), stop=(kc == 1))
            nc.scalar.copy(out=dst_t[:, nch * 512:(nch + 1) * 512], in_=pp[...])

    # ---------- attention ----------
    dif = wk.tile([HID, E], F32)
    nc.vector.scalar_tensor_tensor(out=dif[...], in0=qTe[...], scalar=1e-6,
                                   in1=kkT[...], op0=ALU.add, op1=ALU.subtract)
    sqd = wk.tile([HID, E], BF16)
    nc.scalar.activation(out=sqd[...], in_=dif[...], func=AF.Square)
    M4 = dbc("M1", H)
    Pm = wk.tile([H, E], BF16)
    for nch in range(2):
        sl = slice(nch * 512, (nch + 1) * 512)
        pp = pt("pc", H)
        nc.tensor.matmul(pp[:H, :], hredb[...], sqd[:, sl], start=True, stop=True)
        sdt = wk.tile([H, 512], F32, tag="sdt", name="sdt")
        nc.scalar.activation(out=sdt[...], in_=pp[:H, :], func=AF.Sqrt)
        pe_ = wk.tile([H, 512], F32, tag="pe_", name="pe_")
        nc.scalar.activation(out=pe_[...], in_=sdt[...], func=AF.Exp, scale=-SCALE)
        nc.vector.tensor_tensor(out=Pm[:, sl], in0=pe_[...], in1=M4[:, sl], op=ALU.mult)
    Ssum = wk.tile([H, PCORE], F32)
    nc.vector.tensor_reduce(out=Ssum[...],
                            in_=Pm[...].rearrange("h (j k) -> h j k", k=K),
                            axis=mybir.AxisListType.X, op=ALU.add)
    Rinv = wk.tile([H, PCORE], F32)
    nc.vector.reciprocal(out=Rinv[...], in_=Ssum[...])
    Rinvb = wk.tile([H, PCORE], BF16)
    nc.vector.tensor_copy(out=Rinvb[...], in_=Rinv[...])
    ow = wk.tile([HID, PCORE], F32)
    for nch in range(2):
        sl = slice(nch * 512, (nch + 1) * 512)
        pp = pt("pd")
        nc.tensor.matmul(pp[...], hexpb[...], Pm[:, sl], start=True, stop=True)
        wv = wk.tile([HID, 512], F32, tag="wv", name="wv")
        nc.vector.tensor_tensor(out=wv[...], in0=pp[...], in1=vvT[:, sl], op=ALU.mult)
        nc.vector.tensor_reduce(out=ow[:, nch * 32:(nch + 1) * 32],
                                in_=wv[...].rearrange("c (j k) -> c j k", k=K),
                                axis=mybir.AxisListType.X, op=ALU.add)
    rfp = pt("pc")
    nc.tensor.matmul(rfp[:, :PCORE], hexpb[...], Rinvb[...], start=True, stop=True)
    oT = wk.tile([HID, PCORE], F32)
    nc.vector.tensor_tensor(out=oT[...], in0=ow[...], in1=rfp[:, :PCORE], op=ALU.mult)
    ofp = pt("pd")
    nc.tensor.matmul(ofp[:D, :PCORE], Wout[...], oT[...], start=True, stop=True)
    outFT = wk.tile([D, PCORE], F32)
    nc.scalar.copy(out=outFT[...], in_=ofp[:D, :PCORE])
    dst = bass.AP(tensor=out, offset=0, ap=[[1, D], [D, PCORE]])
    nc.sync.dma_start(out=dst, in_=outFT[...])


def _build_nc():
    nc = bacc.Bacc("TRN2", target_bir_lowering=False, debug=False,
                   num_devices=NCORES)
    P = {}
    def inp(name, shape, dt=F32):
        P[name] = nc.declare_dram_parameter(name, list(shape), dt, isOutput=False)
    inp("fT", (D, NB)); inp("nsc", (D, 1))
    inp("Wq", (D, HID), BF16); inp("Wxi", (D, D), BF16)
    inp("WxjI", (D, 2 * D), BF16)
    inp("gnb", (1, E), BF16); inp("gcb", (1, E), BF16)
    inp("rdT", (1, E), BF16); inp("M1", (1, E), BF16)
    inp("W1", (1, RH), BF16); inp("b1", (RH, 1)); inp("g1", (RH, 1))
    inp("W2", (RH, RH), BF16); inp("b2", (RH, 1)); inp("g2", (RH, 1))
    inp("W3sb", (128, 32, KVD), BF16); inp("b3T", (D, KVD), BF16)
    inp("Wkv", (128, 2, KVD), BF16); inp("Wout", (HID, D))
    inp("hredb", (128, H), BF16); inp("hexpb", (H, 128), BF16)
    inp("iota2", (128, NCH)); inp("mm64", (RH, RH), BF16)
    out = nc.declare_dram_parameter("out", [PCORE, D], F32, isOutput=True)
    import contextlib
    with tile.TileContext(nc) as tc:
        with contextlib.ExitStack() as ctx:
            _emit(nc, tc, P, out, ctx)
    nc.finalize()
    return nc


class _Runner:
    """Builds the sharded jit once; subsequent calls reuse the executable."""

    def __init__(self, nc):
        bass2jax.install_neuronx_cc_hook()
        assert nc.dbg_addr is None
        self.nc = nc
        pid = nc.partition_id_tensor
        self.partition_name = pid.name if pid else None
        in_names, out_names, out_avals = [], [], []
        for alloc in nc.m.functions[0].allocations:
            if not isinstance(alloc, mybir.MemoryLocationSet):
                continue
            name = alloc.memorylocations[0].name
            if alloc.kind == "ExternalInput":
                if name != self.partition_name:
                    in_names.append(name)
            elif alloc.kind == "ExternalOutput":
                shape = tuple(alloc.tensor_shape)
                dtype = mybir.dt.np(alloc.dtype)
                out_names.append(name)
                out_avals.append(jax.core.ShapedArray(shape, dtype))
        all_names = tuple(in_names + out_names +
                          ([self.partition_name] if self.partition_name else []))

        def _body(*args):
            operands = list(args)
            if self.partition_name is not None:
                operands.append(bass2jax.partition_id_tensor())
            return tuple(bass2jax._bass_exec_p.bind(
                *operands, out_avals=tuple(out_avals), in_names=all_names,
                out_names=tuple(out_names), lowering_input_output_aliases=(),
                sim_require_finite=True, sim_require_nnan=True, nc=nc))

        devices = jax.devices()[:NCORES]
        self.mesh = Mesh(np.asarray(devices), ("core",))
        self.sharding = NamedSharding(self.mesh, PartitionSpec("core"))
        n_in, n_out = len(in_names), len(out_names)
        self.fn = jax.jit(
            shard_map(_body, mesh=self.mesh,
                      in_specs=(PartitionSpec("core"),) * (n_in + n_out),
                      out_specs=(PartitionSpec("core"),) * n_out,
                      check_rep=False),
            donate_argnums=tuple(range(n_in, n_in + n_out)),
            keep_unused=True)
        self.in_names = in_names
        self.out_avals = out_avals

    def put(self, a):
        return jax.device_put(np.ascontiguousarray(a), self.sharding)

    def rep(self, a):
        """Replicate a per-core operand into the global (8*rows, ...) layout."""
        a = np.ascontiguousarray(a)
        g = np.broadcast_to(a[None], (NCORES,) + a.shape)
        return self.put(g.reshape(NCORES * a.shape[0], *a.shape[1:]))

    def dispatch(self, by_name):
        args = [by_name[n] for n in self.in_names]
        zeros = [np.zeros((NCORES * av.shape[0], *av.shape[1:]), av.dtype)
                 for av in self.out_avals]
        return self.fn(*args, *zeros)

    def run(self, by_name):
        return np.asarray(self.dispatch(by_name)[0])


_STATE = None


def _init_state():
    bf = ml_dtypes.bfloat16
    nc = _build_nc()
    r = _Runner(nc)

    hred = np.zeros((128, H), bf)
    for h in range(H):
        hred[h * 32:(h + 1) * 32, h] = 1
    hexp = np.ascontiguousarray(hred.T)
    iota2 = (np.arange(128, dtype=np.float32)[:, None]
             + 128.0 * np.arange(NCH, dtype=np.float32)[None, :])
    # center index row differs per core: within-batch node id repeated K times
    gcb = np.empty((NCORES, E), bf)
    for c in range(NCORES):
        loc = (c % (NCORES // B)) * PCORE + np.arange(PCORE)
        gcb[c] = np.repeat(loc, K).astype(bf)

    const = dict(
        hredb=r.rep(hred), hexpb=r.rep(hexp),
        iota2=r.rep(iota2), gcb=r.put(gcb),
        mm64=r.rep(np.full((RH, RH), 1.0 / RH, bf)),
    )
    return {"runner": r, "const": const, "wkey": None, "wdev": None}


def _weights_key(ws):
    h = hashlib.blake2b(digest_size=16)
    for a in ws:
        a = np.asarray(a)
        h.update(str(a.shape).encode()); h.update(str(a.dtype).encode())
        h.update(np.ascontiguousarray(a).tobytes())
    return h.digest()


def _prep_weights(r, norm_scale, Wq, Wxi, Wxj, rp_W1, rp_b1, rp_g1,
                  rp_W2, rp_b2, rp_g2, rp_W3, rp_b3, Wkv_out, Wout):
    bf = ml_dtypes.bfloat16
    WxjI = np.concatenate([np.asarray(Wxj, np.float32),
                           np.eye(D, dtype=np.float32)], axis=1)
    W3sb = np.ascontiguousarray(
        np.asarray(rp_W3, np.float32)
        .reshape(RH, KVD, D).transpose(0, 2, 1)       # (r, d, o)
        .reshape(RH * D, KVD)                         # row = r*64 + d
        .reshape(32, 128, KVD).transpose(1, 0, 2)     # (p, chunk, o)
    ).astype(bf)
    b3T = np.ascontiguousarray(
        np.asarray(rp_b3, np.float32).reshape(KVD, D).T).astype(bf)
    WkvP = np.ascontiguousarray(
        np.asarray(Wkv_out, np.float32).reshape(2, 128, KVD).transpose(1, 0, 2))
    return dict(
        nsc=r.rep(np.asarray(norm_scale, np.float32).reshape(D, 1)),
        Wq=r.rep(np.asarray(Wq, np.float32).astype(bf)),
        Wxi=r.rep(np.asarray(Wxi, np.float32).astype(bf)),
        WxjI=r.rep(WxjI.astype(bf)),
        W1=r.rep(np.asarray(rp_W1, np.float32).reshape(1, RH).astype(bf)),
        b1=r.rep(np.asarray(rp_b1, np.float32).reshape(RH, 1)),
        g1=r.rep(np.asarray(rp_g1, np.float32).reshape(RH, 1)),
        W2=r.rep(np.asarray(rp_W2, np.float32).astype(bf)),
        b2=r.rep(np.asarray(rp_b2, np.float32).reshape(RH, 1)),
        g2=r.rep(np.asarray(rp_g2, np.float32).reshape(RH, 1)),
        W3sb=r.rep(W3sb), b3T=r.rep(b3T), Wkv=r.rep(WkvP.astype(bf)),
        Wout=r.rep(np.asarray(Wout, np.float32)),
    )


def kernel(features, neighbor_indices, neighbor_mask, rel_dist, norm_scale,
           Wq, Wxi, Wxj, rp_W1, rp_b1, rp_g1, rp_W2, rp_b2, rp_g2,
           rp_W3, rp_b3, Wkv_out, Wout):
    global _STATE
    if _STATE is None:
        _STATE = _init_state()
    st = _STATE
    r = st["runner"]

    wlist = (norm_scale, Wq, Wxi, Wxj, rp_W1, rp_b1, rp_g1, rp_W2, rp_b2,
             rp_g2, rp_W3, rp_b3, Wkv_out, Wout)

    f = np.asarray(features, np.float32)
    idx = np.asarray(neighbor_indices)
    msk = np.asarray(neighbor_mask)
    rd = np.asarray(rel_dist, np.float32)

    # per-core activations, laid out as the global (8*rows, ...) arrays
    fb = np.ascontiguousarray(f[..., 0].transpose(0, 2, 1))       # (B, D, N)
    fT = np.broadcast_to(fb[:, None], (B, NCORES // B, D, NB)) \
           .reshape(NCORES * D, NB)                               # core c -> batch c//4
    bf = ml_dtypes.bfloat16
    acts = dict(fT=np.ascontiguousarray(fT),
                gnb=idx.reshape(NCORES, E).astype(bf),
                rdT=rd.reshape(NCORES, E).astype(bf),
                M1=msk.reshape(NCORES, E).astype(bf))

    if st["wkey"] is not None:
        # optimistic: launch with the cached weights, hash while it flies
        outs = r.dispatch({**st["const"], **st["wdev"], **acts})
        wkey = _weights_key(wlist)
        if wkey == st["wkey"]:
            out = np.asarray(outs[0])
            return out.reshape(B, N, D, 1).astype(np.float32)
        del outs  # weights changed: discard the speculative launch
    else:
        wkey = _weights_key(wlist)
    st["wdev"] = _prep_weights(r, *wlist)
    st["wkey"] = wkey
    out = r.run({**st["const"], **st["wdev"], **acts})            # (8*PCORE, D)
    return out.reshape(B, N, D, 1).astype(np.float32)


# revision 32
# speedup vs baseline: 1.1323x; 1.1323x over previous
"""L2-distance attention (degree-0 DTP block) on 8 Trainium2 NeuronCores.

Sharding: 512 (b,n) nodes split 64 per core -> 1024 edges per core; cores
0-3 serve batch 0, cores 4-7 batch 1, so each core only needs its batch's
256 node features. Layout: channel-major (feature channels on SBUF
partitions, edges on the free dim). Neighbor/center gathers are one-hot
selector matmuls; the selectors are built ON DEVICE from the uploaded
index rows (partition-broadcast matmul + is_equal against an iota column),
so the per-call host->device traffic is just features + indices + mask +
distances (~0.6 MB). All weight-derived operands are device-resident and
cached across calls keyed on a content hash; the jit executable is built
once. The per-edge radial contraction kv[o,e] = sum_{r,d} W3[r,o,d]*
hdd[r,e]*xe[d,e] runs as a bf16 GEMM against the Khatri-Rao factor
xs[(r,d),e] accumulated over 32 K-chunks in PSUM.
"""
import hashlib
import numpy as np
import ml_dtypes

import jax
from jax.sharding import Mesh, PartitionSpec, NamedSharding
from jax.experimental.shard_map import shard_map

import concourse.bacc as bacc
import concourse.bass as bass
import concourse.tile as tile
from concourse import mybir, bass2jax

F32 = mybir.dt.float32
BF16 = mybir.dt.bfloat16
AF = mybir.ActivationFunctionType
ALU = mybir.AluOpType

B, N, K, D = 2, 256, 16, 64
H, HID = 4, 128
KVD = 2 * HID
RH = 64
NCORES = 8
NODES = B * N                 # 512
PCORE = NODES // NCORES       # 64 nodes/core
E = PCORE * K                 # 1024 edges/core
NB = N                        # per-batch node space a core needs (256)
NCH = NB // 128               # 2 selector chunks
SCALE = (HID // H) ** -0.5


def _emit(nc, tc, P, out, ctx):
    cst = ctx.enter_context(tc.tile_pool(name="cst", bufs=1))
    wk = ctx.enter_context(tc.tile_pool(name="wk", bufs=1))
    lp = ctx.enter_context(tc.tile_pool(name="lp", bufs=3))
    ps = ctx.enter_context(tc.tile_pool(name="ps", bufs=1, space="PSUM"))
    dr = ctx.enter_context(tc.tile_pool(name="dr", bufs=1, space="DRAM"))

    def load(name, dt=F32):
        t = cst.tile(list(P[name].shape), dt, tag=name)
        nc.sync.dma_start(out=t[...], in_=P[name].ap())
        return t

    # DMA order = need order: tiny early-phase tensors first so the selector
    # build and prenorm aren't stuck behind the 2MB W3sb stream.
    iota2 = load("iota2")
    fT = load("fT"); nsc = load("nsc"); mm64 = load("mm64", BF16)
    rdT = load("rdT", BF16)
    W1 = load("W1", BF16); b1 = load("b1"); g1 = load("g1")
    W2 = load("W2", BF16); b2 = load("b2"); g2 = load("g2")
    Wq = load("Wq", BF16); Wxi = load("Wxi", BF16)
    WxjI = load("WxjI", BF16)
    W3t = load("W3sb", BF16); b3T = load("b3T", BF16)
    Wkv = load("Wkv", BF16); Wout = load("Wout")
    hredb = load("hredb", BF16); hexpb = load("hexpb", BF16)

    eps64 = cst.tile([RH, 1], F32); nc.vector.memset(eps64[...], 1e-5)

    def pt(tag, p=128, w=512):
        return ps.tile([p, w], F32, tag=tag, name=tag)

    def dbc(param, parts, dt=BF16):
        """DMA-broadcast a [1, E] DRAM input across `parts` partitions."""
        t = cst.tile([parts, E], dt, tag="bc_" + param)
        src = bass.AP(tensor=P[param], offset=0, ap=[[0, parts], [1, E]])
        nc.sync.dma_start(out=t[...], in_=src)
        return t

    # ---------------- selectors built on device from index rows -------------
    # Sg[p, ch, e] = 1 iff neighbor(e) == p + 128*ch   (bf16 one-hot);
    # the index row is partition-broadcast by DMA (stride-0 source AP), so
    # no TensorE work is spent here.
    gnB = dbc("gnb", 128)
    gcB = dbc("gcb", 128)
    Sg = wk.tile([128, NCH, E], BF16)
    Cg = wk.tile([128, NCH, E], BF16)
    for src, dstT in ((gnB, Sg), (gcB, Cg)):
        for ch in range(NCH):
            nc.vector.tensor_scalar(dstT[:, ch, :], src[...],
                                    iota2[:, ch:ch + 1], None, ALU.is_equal)

    # -------- prenorm: xT = fT * rsqrt(max(mean f^2, eps)) * norm_scale -----
    # stats broadcast to all D partitions via the (1/64)-ones matmul, so
    # every elementwise op below is partition-parallel (no [1,N] tensors)
    sqf = wk.tile([D, NB], BF16)
    nc.scalar.activation(out=sqf[...], in_=fT[...], func=AF.Square)
    SB = pt("pe", D)
    nc.tensor.matmul(SB[:D, :NB], mm64[:D, :D], sqf[...], start=True, stop=True)
    mx = wk.tile([D, NB], F32)
    nc.vector.tensor_scalar_max(out=mx[...], in0=SB[:D, :NB], scalar1=1e-24)
    sdB = wk.tile([D, NB], F32)
    nc.scalar.activation(out=sdB[...], in_=mx[...], func=AF.Sqrt)
    rinvB = wk.tile([D, NB], F32)
    nc.vector.reciprocal_approx_fast(out=rinvB[...], in_=sdB[...])
    xt1 = wk.tile([D, NB], F32)
    nc.vector.tensor_tensor(out=xt1[...], in0=fT[...], in1=rinvB[...], op=ALU.mult)
    xT = wk.tile([D, NB], BF16)
    nc.vector.tensor_scalar_mul(out=xT[...], in0=xt1[...], scalar1=nsc[...])

    # ---------- node-major chunks: [x@Wxj | x] via one matmul per chunk ------
    x_nm, xj_nm = [], []
    for ch in range(NCH):
        pp = pt("pe")
        nc.tensor.matmul(pp[:, :2 * D], xT[:, ch * 128:(ch + 1) * 128],
                         WxjI[...], start=True, stop=True)
        xj = wk.tile([128, D], BF16, tag=f"xj{ch}", name=f"xj{ch}")
        nc.scalar.copy(out=xj[...], in_=pp[:, :D])
        xn = wk.tile([128, D], BF16, tag=f"xn{ch}", name=f"xn{ch}")
        nc.scalar.copy(out=xn[...], in_=pp[:, D:2 * D])
        xj_nm.append(xj); x_nm.append(xn)

    # ---------- center replicate: xTe[d, e] = x[ctr(e), d] ----------
    xTe = wk.tile([D, E], BF16)
    for nch in range(2):
        pp = pt("pe" if nch == 0 else "pf", D)
        for ch in range(NCH):
            nc.tensor.matmul(pp[:D, :], x_nm[ch][...],
                             Cg[:, ch, nch * 512:(nch + 1) * 512],
                             start=(ch == 0), stop=(ch == NCH - 1))
        nc.scalar.copy(out=xTe[:, nch * 512:(nch + 1) * 512], in_=pp[:D, :])

    # ---------- edge features: xeT = xg(neighbor) + xi(center) ----------
    xeT_ps = []
    for nch in range(2):
        pp = pt("pa" if nch == 0 else "pb", D)
        xeT_ps.append(pp)
        for ch in range(NCH):
            nc.tensor.matmul(pp[:D, :], xj_nm[ch][...],
                             Sg[:, ch, nch * 512:(nch + 1) * 512],
                             start=(ch == 0), stop=False)
        nc.tensor.matmul(pp[:D, :], Wxi[...],
                         xTe[:, nch * 512:(nch + 1) * 512],
                         start=False, stop=True)
    stack = wk.tile([128, E], BF16)   # [xeT; xeT] bf16
    for nch in range(2):
        sl = slice(nch * 512, (nch + 1) * 512)
        nc.vector.tensor_copy(out=stack[:D, sl], in_=xeT_ps[nch][:D, :])
        nc.scalar.copy(out=stack[D:, sl], in_=xeT_ps[nch][:D, :])

    # ---------- queries per edge ----------
    qTe = wk.tile([HID, E], F32)
    for nch in range(2):
        pp = pt("pc")
        nc.tensor.matmul(pp[...], Wq[...], xTe[:, nch * 512:(nch + 1) * 512],
                         start=True, stop=True)
        nc.scalar.copy(out=qTe[:, nch * 512:(nch + 1) * 512], in_=pp[...])

    # ---------- radial MLP: 2 x (linear -> silu -> LN*g), channel-major ------
    # LN stats are broadcast to all RH partitions directly by the (1/64)-ones
    # matmul (muB = mean, q2B = E[z^2]), so the whole pipeline stays
    # partition-parallel with no [1,E] intermediates.
    def radial_layer(z_src_ps, bias, g, out_dt, tg):
        z = wk.tile([RH, E], BF16, tag=tg + "z", name=tg + "z")
        for nch in range(2):
            nc.scalar.activation(out=z[:, nch * 512:(nch + 1) * 512],
                                 in_=z_src_ps[nch][:RH, :], func=AF.Silu,
                                 bias=bias[...], scale=1.0)
        sq = wk.tile([RH, E], BF16, tag=tg + "q", name=tg + "q")
        nc.scalar.activation(out=sq[...], in_=z[...], func=AF.Square)
        hddo = wk.tile([RH, E], out_dt, tag=tg)
        for nch in range(2):
            sl = slice(nch * 512, (nch + 1) * 512)
            muB = pt("pc", RH)
            nc.tensor.matmul(muB[:RH, :], mm64[...], z[:, sl], start=True, stop=True)
            q2B = pt("pd", RH)
            nc.tensor.matmul(q2B[:RH, :], mm64[...], sq[:, sl], start=True, stop=True)
            m2 = wk.tile([RH, 512], F32, tag=tg + "m2", name=tg + "m2")
            nc.scalar.activation(out=m2[...], in_=muB[:RH, :], func=AF.Square)
            var = wk.tile([RH, 512], F32, tag=tg + "v", name=tg + "v")
            nc.vector.tensor_tensor(out=var[...], in0=q2B[:RH, :], in1=m2[...], op=ALU.subtract)
            sd = wk.tile([RH, 512], F32, tag=tg + "sd", name=tg + "sd")
            nc.scalar.activation(out=sd[...], in_=var[...], func=AF.Sqrt,
                                 bias=eps64[...], scale=1.0)  # sqrt(var+eps)
            rstd = wk.tile([RH, 512], F32, tag=tg + "rs", name=tg + "rs")
            nc.vector.reciprocal_approx_fast(out=rstd[...], in_=sd[...])
            d1 = wk.tile([RH, 512], F32, tag=tg + "d1", name=tg + "d1")
            nc.vector.tensor_tensor(out=d1[...], in0=z[:, sl], in1=muB[:RH, :], op=ALU.subtract)
            d2 = wk.tile([RH, 512], F32, tag=tg + "d2", name=tg + "d2")
            nc.vector.tensor_tensor(out=d2[...], in0=d1[...], in1=rstd[...], op=ALU.mult)
            nc.vector.tensor_scalar_mul(out=hddo[:, sl], in0=d2[...], scalar1=g[...])
        return hddo

    h1ps = []
    for nch in range(2):
        pp = pt("pe" if nch == 0 else "pf", RH)
        nc.tensor.matmul(pp[:RH, :], W1[...], rdT[:, nch * 512:(nch + 1) * 512],
                         start=True, stop=True)
        h1ps.append(pp)
    hdd1 = radial_layer(h1ps, b1, g1, BF16, "h1")
    h2ps = []
    for nch in range(2):
        pp = pt("pe" if nch == 0 else "pf", RH)
        nc.tensor.matmul(pp[:RH, :], W2[...], hdd1[:, nch * 512:(nch + 1) * 512],
                         start=True, stop=True)
        h2ps.append(pp)
    hddT = radial_layer(h2ps, b2, g2, BF16, "h2")

    # ---------- big GEMM: kv[o,e] = sum_{rd} W3'[rd,o] * xs[rd,e] ----------
    # The per-chunk hdd-row replication runs on the DMA engines (hddT is
    # bounced through DRAM once, then each chunk's two rows are read back
    # with a stride-0 partition AP), so TensorE spends rows only on the
    # 4 accumulation matmuls per chunk. DMA -> DVE -> PE pipeline is 3-deep
    # via the lp pool rotation.
    hdram = dr.tile([RH, E], BF16)
    nc.sync.dma_start(out=hdram[...], in_=hddT[...])
    # prefetch ALL replicated hdd rows into SBUF up front (4 groups of 8
    # chunks so early groups unblock the loop while later ones stream);
    # the idle gpsimd queue issues them, leaving sync for everything else
    hA = [cst.tile([128, 8, E], BF16, tag=f"hA{g}", name=f"hA{g}")
          for g in range(4)]
    for g in range(4):
        for cc in range(8):
            for half in range(2):
                r = 2 * (g * 8 + cc) + half
                nc.gpsimd.dma_start(out=hA[g][half * RH:(half + 1) * RH, cc, :],
                                    in_=hdram[r:r + 1, :].broadcast_to([RH, E]))
    kvtags = ["pa", "pb", "pc", "pd"]
    kvps = [[pt(kvtags[2 * m + n]) for n in range(2)] for m in range(2)]
    for c in range(32):
        xs = lp.tile([128, E], BF16, tag="xs", name="xs")
        nc.vector.tensor_tensor(out=xs[...], in0=stack[...],
                                in1=hA[c // 8][:, c % 8, :], op=ALU.mult)
        for m in range(2):
            for nch in range(2):
                nc.tensor.matmul(kvps[m][nch][...],
                                 W3t[:, c, m * 128:(m + 1) * 128],
                                 xs[:, nch * 512:(nch + 1) * 512],
                                 start=(c == 0), stop=False)
    for m in range(2):
        for nch in range(2):
            nc.tensor.matmul(kvps[m][nch][...], b3T[:, m * 128:(m + 1) * 128],
                             stack[:D, nch * 512:(nch + 1) * 512],
                             start=False, stop=True)
    kvT = wk.tile([128, 2, E], BF16)
    for m in range(2):
        for nch in range(2):
            nc.scalar.copy(out=kvT[:, m, nch * 512:(nch + 1) * 512],
                           in_=kvps[m][nch][...])

    # ---------- kv2 = Wkv^T @ kv : kk rows 0:128, vv rows 128:256 ----------
    kkT = wk.tile([HID, E], F32)
    vvT = wk.tile([HID, E], F32)
    for m, dst_t in ((0, kkT), (1, vvT)):
        for nch in range(2):
            pp = pt("pa" if nch == 0 else "pb")
            for kc in range(2):
                nc.tensor.matmul(pp[...],
                                 Wkv[:, kc, m * 128:(m + 1) * 128],
                                 kvT[:, kc, nch * 512:(nch + 1) * 512],
                                 start=(kc == 0), stop=(kc == 1))
            nc.scalar.copy(out=dst_t[:, nch * 512:(nch + 1) * 512], in_=pp[...])

    # ---------- attention ----------
    dif = wk.tile([HID, E], F32)
    nc.vector.scalar_tensor_tensor(out=dif[...], in0=qTe[...], scalar=1e-6,
                                   in1=kkT[...], op0=ALU.add, op1=ALU.subtract)
    sqd = wk.tile([HID, E], BF16)
    nc.scalar.activation(out=sqd[...], in_=dif[...], func=AF.Square)
    M4 = dbc("M1", H)
    Pm = wk.tile([H, E], BF16)
    for nch in range(2):
        sl = slice(nch * 512, (nch + 1) * 512)
        pp = pt("pc", H)
        nc.tensor.matmul(pp[:H, :], hredb[...], sqd[:, sl], start=True, stop=True)
        sdt = wk.tile([H, 512], F32, tag="sdt", name="sdt")
        nc.scalar.activation(out=sdt[...], in_=pp[:H, :], func=AF.Sqrt)
        pe_ = wk.tile([H, 512], F32, tag="pe_", name="pe_")
        nc.scalar.activation(out=pe_[...], in_=sdt[...], func=AF.Exp, scale=-SCALE)
        nc.vector.tensor_tensor(out=Pm[:, sl], in0=pe_[...], in1=M4[:, sl], op=ALU.mult)
    Ssum = wk.tile([H, PCORE], F32)
    nc.vector.tensor_reduce(out=Ssum[...],
                            in_=Pm[...].rearrange("h (j k) -> h j k", k=K),
                            axis=mybir.AxisListType.X, op=ALU.add)
    Rinv = wk.tile([H, PCORE], F32)
    nc.vector.reciprocal(out=Rinv[...], in_=Ssum[...])
    Rinvb = wk.tile([H, PCORE], BF16)
    nc.vector.tensor_copy(out=Rinvb[...], in_=Rinv[...])
    ow = wk.tile([HID, PCORE], F32)
    for nch in range(2):
        sl = slice(nch * 512, (nch + 1) * 512)
        pp = pt("pd")
        nc.tensor.matmul(pp[...], hexpb[...], Pm[:, sl], start=True, stop=True)
        wv = wk.tile([HID, 512], F32, tag="wv", name="wv")
        nc.vector.tensor_tensor(out=wv[...], in0=pp[...], in1=vvT[:, sl], op=ALU.mult)
        nc.vector.tensor_reduce(out=ow[:, nch * 32:(nch + 1) * 32],
                                in_=wv[...].rearrange("c (j k) -> c j k", k=K),
                                axis=mybir.AxisListType.X, op=ALU.add)
    rfp = pt("pc")
    nc.tensor.matmul(rfp[:, :PCORE], hexpb[...], Rinvb[...], start=True, stop=True)
    oT = wk.tile([HID, PCORE], F32)
    nc.vector.tensor_tensor(out=oT[...], in0=ow[...], in1=rfp[:, :PCORE], op=ALU.mult)
    ofp = pt("pd")
    nc.tensor.matmul(ofp[:D, :PCORE], Wout[...], oT[...], start=True, stop=True)
    outFT = wk.tile([D, PCORE], F32)
    nc.scalar.copy(out=outFT[...], in_=ofp[:D, :PCORE])
    dst = bass.AP(tensor=out, offset=0, ap=[[1, D], [D, PCORE]])
    nc.sync.dma_start(out=dst, in_=outFT[...])


def _build_nc():
    nc = bacc.Bacc("TRN2", target_bir_lowering=False, debug=False,
                   num_devices=NCORES)
    P = {}
    def inp(name, shape, dt=F32):
        P[name] = nc.declare_dram_parameter(name, list(shape), dt, isOutput=False)
    inp("fT", (D, NB)); inp("nsc", (D, 1))
    inp("Wq", (D, HID), BF16); inp("Wxi", (D, D), BF16)
    inp("WxjI", (D, 2 * D), BF16)
    inp("gnb", (1, E), BF16); inp("gcb", (1, E), BF16)
    inp("rdT", (1, E), BF16); inp("M1", (1, E), BF16)
    inp("W1", (1, RH), BF16); inp("b1", (RH, 1)); inp("g1", (RH, 1))
    inp("W2", (RH, RH), BF16); inp("b2", (RH, 1)); inp("g2", (RH, 1))
    inp("W3sb", (128, 32, KVD), BF16); inp("b3T", (D, KVD), BF16)
    inp("Wkv", (128, 2, KVD), BF16); inp("Wout", (HID, D))
    inp("hredb", (128, H), BF16); inp("hexpb", (H, 128), BF16)
    inp("iota2", (128, NCH)); inp("mm64", (RH, RH), BF16)
    out = nc.declare_dram_parameter("out", [PCORE, D], F32, isOutput=True)
    import contextlib
    with tile.TileContext(nc) as tc:
        with contextlib.ExitStack() as ctx:
            _emit(nc, tc, P, out, ctx)
    nc.finalize()
    return nc


class _Runner:
    """Builds the sharded jit once; subsequent calls reuse the executable."""

    def __init__(self, nc):
        bass2jax.install_neuronx_cc_hook()
        assert nc.dbg_addr is None
        self.nc = nc
        pid = nc.partition_id_tensor
        self.partition_name = pid.name if pid else None
        in_names, out_names, out_avals = [], [], []
        for alloc in nc.m.functions[0].allocations:
            if not isinstance(alloc, mybir.MemoryLocationSet):
                continue
            name = alloc.memorylocations[0].name
            if alloc.kind == "ExternalInput":
                if name != self.partition_name:
                    in_names.append(name)
            elif alloc.kind == "ExternalOutput":
                shape = tuple(alloc.tensor_shape)
                dtype = mybir.dt.np(alloc.dtype)
                out_names.append(name)
                out_avals.append(jax.core.ShapedArray(shape, dtype))
        all_names = tuple(in_names + out_names +
                          ([self.partition_name] if self.partition_name else []))

        def _body(*args):
            operands = list(args)
            if self.partition_name is not None:
                operands.append(bass2jax.partition_id_tensor())
            return tuple(bass2jax._bass_exec_p.bind(
                *operands, out_avals=tuple(out_avals), in_names=all_names,
                out_names=tuple(out_names), lowering_input_output_aliases=(),
                sim_require_finite=True, sim_require_nnan=True, nc=nc))

        devices = jax.devices()[:NCORES]
        self.mesh = Mesh(np.asarray(devices), ("core",))
        self.sharding = NamedSharding(self.mesh, PartitionSpec("core"))
        n_in, n_out = len(in_names), len(out_names)
        self.fn = jax.jit(
            shard_map(_body, mesh=self.mesh,
                      in_specs=(PartitionSpec("core"),) * (n_in + n_out),
                      out_specs=(PartitionSpec("core"),) * n_out,
                      check_rep=False),
            donate_argnums=tuple(range(n_in, n_in + n_out)),
            keep_unused=True)
        self.in_names = in_names
        self.out_avals = out_avals

    def put(self, a):
        return jax.device_put(np.ascontiguousarray(a), self.sharding)

    def rep(self, a):
        """Replicate a per-core operand into the global (8*rows, ...) layout."""
        a = np.ascontiguousarray(a)
        g = np.broadcast_to(a[None], (NCORES,) + a.shape)
        return self.put(g.reshape(NCORES * a.shape[0], *a.shape[1:]))

    def dispatch(self, by_name):
        args = [by_name[n] for n in self.in_names]
        zeros = [np.zeros((NCORES * av.shape[0], *av.shape[1:]), av.dtype)
                 for av in self.out_avals]
        return self.fn(*args, *zeros)

    def run(self, by_name):
        return np.asarray(self.dispatch(by_name)[0])


_STATE = None


def _init_state():
    bf = ml_dtypes.bfloat16
    nc = _build_nc()
    r = _Runner(nc)

    hred = np.zeros((128, H), bf)
    for h in range(H):
        hred[h * 32:(h + 1) * 32, h] = 1
    hexp = np.ascontiguousarray(hred.T)
    iota2 = (np.arange(128, dtype=np.float32)[:, None]
             + 128.0 * np.arange(NCH, dtype=np.float32)[None, :])
    # center index row differs per core: within-batch node id repeated K times
    gcb = np.empty((NCORES, E), bf)
    for c in range(NCORES):
        loc = (c % (NCORES // B)) * PCORE + np.arange(PCORE)
        gcb[c] = np.repeat(loc, K).astype(bf)

    const = dict(
        hredb=r.rep(hred), hexpb=r.rep(hexp),
        iota2=r.rep(iota2), gcb=r.put(gcb),
        mm64=r.rep(np.full((RH, RH), 1.0 / RH, bf)),
    )
    return {"runner": r, "const": const, "wkey": None, "wdev": None}


def _weights_key(ws):
    h = hashlib.blake2b(digest_size=16)
    for a in ws:
        a = np.asarray(a)
        h.update(str(a.shape).encode()); h.update(str(a.dtype).encode())
        h.update(np.ascontiguousarray(a).tobytes())
    return h.digest()


def _prep_weights(r, norm_scale, Wq, Wxi, Wxj, rp_W1, rp_b1, rp_g1,
                  rp_W2, rp_b2, rp_g2, rp_W3, rp_b3, Wkv_out, Wout):
    bf = ml_dtypes.bfloat16
    WxjI = np.concatenate([np.asarray(Wxj, np.float32),
                           np.eye(D, dtype=np.float32)], axis=1)
    W3sb = np.ascontiguousarray(
        np.asarray(rp_W3, np.float32)
        .reshape(RH, KVD, D).transpose(0, 2, 1)       # (r, d, o)
        .reshape(RH * D, KVD)                         # row = r*64 + d
        .reshape(32, 128, KVD).transpose(1, 0, 2)     # (p, chunk, o)
    ).astype(bf)
    b3T = np.ascontiguousarray(
        np.asarray(rp_b3, np.float32).reshape(KVD, D).T).astype(bf)
    WkvP = np.ascontiguousarray(
        np.asarray(Wkv_out, np.float32).reshape(2, 128, KVD).transpose(1, 0, 2))
    return dict(
        nsc=r.rep(np.asarray(norm_scale, np.float32).reshape(D, 1)),
        Wq=r.rep(np.asarray(Wq, np.float32).astype(bf)),
        Wxi=r.rep(np.asarray(Wxi, np.float32).astype(bf)),
        WxjI=r.rep(WxjI.astype(bf)),
        W1=r.rep(np.asarray(rp_W1, np.float32).reshape(1, RH).astype(bf)),
        b1=r.rep(np.asarray(rp_b1, np.float32).reshape(RH, 1)),
        g1=r.rep(np.asarray(rp_g1, np.float32).reshape(RH, 1)),
        W2=r.rep(np.asarray(rp_W2, np.float32).astype(bf)),
        b2=r.rep(np.asarray(rp_b2, np.float32).reshape(RH, 1)),
        g2=r.rep(np.asarray(rp_g2, np.float32).reshape(RH, 1)),
        W3sb=r.rep(W3sb), b3T=r.rep(b3T), Wkv=r.rep(WkvP.astype(bf)),
        Wout=r.rep(np.asarray(Wout, np.float32)),
    )


def kernel(features, neighbor_indices, neighbor_mask, rel_dist, norm_scale,
           Wq, Wxi, Wxj, rp_W1, rp_b1, rp_g1, rp_W2, rp_b2, rp_g2,
           rp_W3, rp_b3, Wkv_out, Wout):
    global _STATE
    if _STATE is None:
        _STATE = _init_state()
    st = _STATE
    r = st["runner"]

    wlist = (norm_scale, Wq, Wxi, Wxj, rp_W1, rp_b1, rp_g1, rp_W2, rp_b2,
             rp_g2, rp_W3, rp_b3, Wkv_out, Wout)

    f = np.asarray(features, np.float32)
    idx = np.asarray(neighbor_indices)
    msk = np.asarray(neighbor_mask)
    rd = np.asarray(rel_dist, np.float32)

    # per-core activations, laid out as the global (8*rows, ...) arrays
    fb = np.ascontiguousarray(f[..., 0].transpose(0, 2, 1))       # (B, D, N)
    fT = np.broadcast_to(fb[:, None], (B, NCORES // B, D, NB)) \
           .reshape(NCORES * D, NB)                               # core c -> batch c//4
    bf = ml_dtypes.bfloat16
    acts = dict(fT=np.ascontiguousarray(fT),
                gnb=idx.reshape(NCORES, E).astype(bf),
                rdT=rd.reshape(NCORES, E).astype(bf),
                M1=msk.reshape(NCORES, E).astype(bf))

    if st["wkey"] is not None:
        # optimistic: launch with the cached weights, hash while it flies
        outs = r.dispatch({**st["const"], **st["wdev"], **acts})
        wkey = _weights_key(wlist)
        if wkey == st["wkey"]:
            out = np.asarray(outs[0])
            return out.reshape(B, N, D, 1).astype(np.float32)
        del outs  # weights changed: discard the speculative launch
    else:
        wkey = _weights_key(wlist)
    st["wdev"] = _prep_weights(r, *wlist)
    st["wkey"] = wkey
    out = r.run({**st["const"], **st["wdev"], **acts})            # (8*PCORE, D)
    return out.reshape(B, N, D, 1).astype(np.float32)


# revision 33
# speedup vs baseline: 1.2289x; 1.0853x over previous
"""L2-distance attention (degree-0 DTP block) on 8 Trainium2 NeuronCores.

Sharding: 512 (b,n) nodes split 64 per core -> 1024 edges per core; cores
0-3 serve batch 0, cores 4-7 batch 1, so each core only needs its batch's
256 node features. Layout: channel-major (feature channels on SBUF
partitions, edges on the free dim). Neighbor/center gathers are one-hot
selector matmuls; the selectors are built ON DEVICE from the uploaded
index rows (partition-broadcast matmul + is_equal against an iota column),
so the per-call host->device traffic is just features + indices + mask +
distances (~0.6 MB). All weight-derived operands are device-resident and
cached across calls keyed on a content hash; the jit executable is built
once. The per-edge radial contraction kv[o,e] = sum_{r,d} W3[r,o,d]*
hdd[r,e]*xe[d,e] runs as a bf16 GEMM against the Khatri-Rao factor
xs[(r,d),e] accumulated over 32 K-chunks in PSUM.
"""
import hashlib
import numpy as np
import ml_dtypes

import jax
from jax.sharding import Mesh, PartitionSpec, NamedSharding
from jax.experimental.shard_map import shard_map

import concourse.bacc as bacc
import concourse.bass as bass
import concourse.tile as tile
from concourse import mybir, bass2jax

F32 = mybir.dt.float32
BF16 = mybir.dt.bfloat16
AF = mybir.ActivationFunctionType
ALU = mybir.AluOpType

B, N, K, D = 2, 256, 16, 64
H, HID = 4, 128
KVD = 2 * HID
RH = 64
NCORES = 8
NODES = B * N                 # 512
PCORE = NODES // NCORES       # 64 nodes/core
E = PCORE * K                 # 1024 edges/core
NB = N                        # per-batch node space a core needs (256)
NCH = NB // 128               # 2 selector chunks
SCALE = (HID // H) ** -0.5


def _emit(nc, tc, P, out, ctx):
    cst = ctx.enter_context(tc.tile_pool(name="cst", bufs=1))
    wk = ctx.enter_context(tc.tile_pool(name="wk", bufs=1))
    lp = ctx.enter_context(tc.tile_pool(name="lp", bufs=3))
    ps = ctx.enter_context(tc.tile_pool(name="ps", bufs=1, space="PSUM"))
    dr = ctx.enter_context(tc.tile_pool(name="dr", bufs=1, space="DRAM"))

    def load(name, dt=F32):
        t = cst.tile(list(P[name].shape), dt, tag=name)
        nc.sync.dma_start(out=t[...], in_=P[name].ap())
        return t

    # DMA order = need order: tiny early-phase tensors first so the selector
    # build and prenorm aren't stuck behind the 2MB W3sb stream.
    iota2 = load("iota2")
    fT = load("fT"); nsc = load("nsc"); mm64 = load("mm64", BF16)
    rdT = load("rdT", BF16)
    W1 = load("W1", BF16); b1 = load("b1"); g1 = load("g1")
    W2 = load("W2", BF16); b2 = load("b2"); g2 = load("g2")
    Wq = load("Wq", BF16); Wxi = load("Wxi", BF16)
    WxjI = load("WxjI", BF16)
    W3t = load("W3sb", BF16); b3T = load("b3T", BF16)
    Wkv = load("Wkv", BF16); Wout = load("Wout")
    hredb = load("hredb", BF16); hexpb = load("hexpb", BF16)

    eps64 = cst.tile([RH, 1], F32); nc.vector.memset(eps64[...], 1e-5)

    def pt(tag, p=128, w=512):
        return ps.tile([p, w], F32, tag=tag, name=tag)

    def dbc(param, parts, dt=BF16):
        """DMA-broadcast a [1, E] DRAM input across `parts` partitions."""
        t = cst.tile([parts, E], dt, tag="bc_" + param)
        src = bass.AP(tensor=P[param], offset=0, ap=[[0, parts], [1, E]])
        nc.sync.dma_start(out=t[...], in_=src)
        return t

    # ---------------- selectors built on device from index rows -------------
    # Sg[p, ch, e] = 1 iff neighbor(e) == p + 128*ch   (bf16 one-hot);
    # the index row is partition-broadcast by DMA (stride-0 source AP), so
    # no TensorE work is spent here.
    gnB = dbc("gnb", 128)
    gcB = dbc("gcb", 128)
    Sg = wk.tile([128, NCH, E], BF16)
    Cg = wk.tile([128, NCH, E], BF16)
    for src, dstT in ((gnB, Sg), (gcB, Cg)):
        for ch in range(NCH):
            nc.vector.tensor_scalar(dstT[:, ch, :], src[...],
                                    iota2[:, ch:ch + 1], None, ALU.is_equal)

    # -------- prenorm: xT = fT * rsqrt(max(mean f^2, eps)) * norm_scale -----
    # stats broadcast to all D partitions via the (1/64)-ones matmul, so
    # every elementwise op below is partition-parallel (no [1,N] tensors)
    sqf = wk.tile([D, NB], BF16)
    nc.scalar.activation(out=sqf[...], in_=fT[...], func=AF.Square)
    SB = pt("pe", D)
    nc.tensor.matmul(SB[:D, :NB], mm64[:D, :D], sqf[...], start=True, stop=True)
    mx = wk.tile([D, NB], F32)
    nc.vector.tensor_scalar_max(out=mx[...], in0=SB[:D, :NB], scalar1=1e-24)
    sdB = wk.tile([D, NB], F32)
    nc.scalar.activation(out=sdB[...], in_=mx[...], func=AF.Sqrt)
    rinvB = wk.tile([D, NB], F32)
    nc.vector.reciprocal_approx_fast(out=rinvB[...], in_=sdB[...])
    xt1 = wk.tile([D, NB], F32)
    nc.vector.tensor_tensor(out=xt1[...], in0=fT[...], in1=rinvB[...], op=ALU.mult)
    xT = wk.tile([D, NB], BF16)
    nc.vector.tensor_scalar_mul(out=xT[...], in0=xt1[...], scalar1=nsc[...])

    # ---------- node-major chunks: [x@Wxj | x] via one matmul per chunk ------
    x_nm, xj_nm = [], []
    for ch in range(NCH):
        pp = pt("pe")
        nc.tensor.matmul(pp[:, :2 * D], xT[:, ch * 128:(ch + 1) * 128],
                         WxjI[...], start=True, stop=True)
        xj = wk.tile([128, D], BF16, tag=f"xj{ch}", name=f"xj{ch}")
        nc.scalar.copy(out=xj[...], in_=pp[:, :D])
        xn = wk.tile([128, D], BF16, tag=f"xn{ch}", name=f"xn{ch}")
        nc.scalar.copy(out=xn[...], in_=pp[:, D:2 * D])
        xj_nm.append(xj); x_nm.append(xn)

    # ---------- center replicate: xTe[d, e] = x[ctr(e), d] ----------
    xTe = wk.tile([D, E], BF16)
    for nch in range(2):
        pp = pt("pe" if nch == 0 else "pf", D)
        for ch in range(NCH):
            nc.tensor.matmul(pp[:D, :], x_nm[ch][...],
                             Cg[:, ch, nch * 512:(nch + 1) * 512],
                             start=(ch == 0), stop=(ch == NCH - 1))
        nc.scalar.copy(out=xTe[:, nch * 512:(nch + 1) * 512], in_=pp[:D, :])

    # ---------- edge features: xeT = xg(neighbor) + xi(center) ----------
    xeT_ps = []
    for nch in range(2):
        pp = pt("pa" if nch == 0 else "pb", D)
        xeT_ps.append(pp)
        for ch in range(NCH):
            nc.tensor.matmul(pp[:D, :], xj_nm[ch][...],
                             Sg[:, ch, nch * 512:(nch + 1) * 512],
                             start=(ch == 0), stop=False)
        nc.tensor.matmul(pp[:D, :], Wxi[...],
                         xTe[:, nch * 512:(nch + 1) * 512],
                         start=False, stop=True)
    stack = wk.tile([128, E], BF16)   # [xeT; xeT] bf16
    for nch in range(2):
        sl = slice(nch * 512, (nch + 1) * 512)
        nc.vector.tensor_copy(out=stack[:D, sl], in_=xeT_ps[nch][:D, :])
        nc.scalar.copy(out=stack[D:, sl], in_=xeT_ps[nch][:D, :])

    # ---------- queries per edge ----------
    qTe = wk.tile([HID, E], F32)
    for nch in range(2):
        pp = pt("pc")
        nc.tensor.matmul(pp[...], Wq[...], xTe[:, nch * 512:(nch + 1) * 512],
                         start=True, stop=True)
        nc.scalar.copy(out=qTe[:, nch * 512:(nch + 1) * 512], in_=pp[...])

    # ---------- radial MLP: 2 x (linear -> silu -> LN*g), channel-major ------
    # LN stats are broadcast to all RH partitions directly by the (1/64)-ones
    # matmul (muB = mean, q2B = E[z^2]), so the whole pipeline stays
    # partition-parallel with no [1,E] intermediates.
    def radial_layer(z_src_ps, bias, g, out_dt, tg):
        z = wk.tile([RH, E], BF16, tag=tg + "z", name=tg + "z")
        for nch in range(2):
            nc.scalar.activation(out=z[:, nch * 512:(nch + 1) * 512],
                                 in_=z_src_ps[nch][:RH, :], func=AF.Silu,
                                 bias=bias[...], scale=1.0)
        sq = wk.tile([RH, E], BF16, tag=tg + "q", name=tg + "q")
        nc.scalar.activation(out=sq[...], in_=z[...], func=AF.Square)
        hddo = wk.tile([RH, E], out_dt, tag=tg)
        for nch in range(2):
            sl = slice(nch * 512, (nch + 1) * 512)
            muB = pt("pc", RH)
            nc.tensor.matmul(muB[:RH, :], mm64[...], z[:, sl], start=True, stop=True)
            q2B = pt("pd", RH)
            nc.tensor.matmul(q2B[:RH, :], mm64[...], sq[:, sl], start=True, stop=True)
            m2 = wk.tile([RH, 512], F32, tag=tg + "m2", name=tg + "m2")
            nc.scalar.activation(out=m2[...], in_=muB[:RH, :], func=AF.Square)
            var = wk.tile([RH, 512], F32, tag=tg + "v", name=tg + "v")
            nc.vector.tensor_tensor(out=var[...], in0=q2B[:RH, :], in1=m2[...], op=ALU.subtract)
            sd = wk.tile([RH, 512], F32, tag=tg + "sd", name=tg + "sd")
            nc.scalar.activation(out=sd[...], in_=var[...], func=AF.Sqrt,
                                 bias=eps64[...], scale=1.0)  # sqrt(var+eps)
            rstd = wk.tile([RH, 512], F32, tag=tg + "rs", name=tg + "rs")
            nc.vector.reciprocal_approx_fast(out=rstd[...], in_=sd[...])
            d1 = wk.tile([RH, 512], F32, tag=tg + "d1", name=tg + "d1")
            nc.vector.tensor_tensor(out=d1[...], in0=z[:, sl], in1=muB[:RH, :], op=ALU.subtract)
            d2 = wk.tile([RH, 512], F32, tag=tg + "d2", name=tg + "d2")
            nc.vector.tensor_tensor(out=d2[...], in0=d1[...], in1=rstd[...], op=ALU.mult)
            nc.vector.tensor_scalar_mul(out=hddo[:, sl], in0=d2[...], scalar1=g[...])
        return hddo

    h1ps = []
    for nch in range(2):
        pp = pt("pe" if nch == 0 else "pf", RH)
        nc.tensor.matmul(pp[:RH, :], W1[...], rdT[:, nch * 512:(nch + 1) * 512],
                         start=True, stop=True)
        h1ps.append(pp)
    hdd1 = radial_layer(h1ps, b1, g1, BF16, "h1")
    h2ps = []
    for nch in range(2):
        pp = pt("pe" if nch == 0 else "pf", RH)
        nc.tensor.matmul(pp[:RH, :], W2[...], hdd1[:, nch * 512:(nch + 1) * 512],
                         start=True, stop=True)
        h2ps.append(pp)
    hddT = radial_layer(h2ps, b2, g2, BF16, "h2")

    # ---------- big GEMM: kv[o,e] = sum_{rd} W3'[rd,o] * xs[rd,e] ----------
    # The per-chunk hdd-row replication runs on the DMA engines (hddT is
    # bounced through DRAM once, then each chunk's two rows are read back
    # with a stride-0 partition AP), so TensorE spends rows only on the
    # 4 accumulation matmuls per chunk. DMA -> DVE -> PE pipeline is 3-deep
    # via the lp pool rotation.
    hdram = dr.tile([RH, E], BF16)
    nc.sync.dma_start(out=hdram[...], in_=hddT[...])
    # prefetch ALL replicated hdd rows into SBUF up front (4 groups of 8
    # chunks so early groups unblock the loop while later ones stream);
    # the idle gpsimd queue issues them, leaving sync for everything else
    hA = [cst.tile([128, 8, E], BF16, tag=f"hA{g}", name=f"hA{g}")
          for g in range(4)]
    hd = hdram[...]
    for g in range(4):
        for half in range(2):
            # rows 2*(8g+cc)+half for cc in 0..8, each replicated across the
            # RH partitions: one strided DMA ([[0,RH],[2E,8],[1,E]] source)
            src = bass.AP(tensor=hd.tensor,
                          offset=hd.offset + (16 * g + half) * E,
                          ap=[[0, RH], [2 * E, 8], [1, E]])
            nc.gpsimd.dma_start(out=hA[g][half * RH:(half + 1) * RH, :, :],
                                in_=src)
    kvtags = ["pa", "pb", "pc", "pd"]
    kvps = [[pt(kvtags[2 * m + n]) for n in range(2)] for m in range(2)]
    for c in range(32):
        xs = lp.tile([128, E], BF16, tag="xs", name="xs")
        nc.vector.tensor_tensor(out=xs[...], in0=stack[...],
                                in1=hA[c // 8][:, c % 8, :], op=ALU.mult)
        for m in range(2):
            for nch in range(2):
                nc.tensor.matmul(kvps[m][nch][...],
                                 W3t[:, c, m * 128:(m + 1) * 128],
                                 xs[:, nch * 512:(nch + 1) * 512],
                                 start=(c == 0), stop=False)
    for m in range(2):
        for nch in range(2):
            nc.tensor.matmul(kvps[m][nch][...], b3T[:, m * 128:(m + 1) * 128],
                             stack[:D, nch * 512:(nch + 1) * 512],
                             start=False, stop=True)
    kvT = wk.tile([128, 2, E], BF16)
    for m in range(2):
        for nch in range(2):
            nc.scalar.copy(out=kvT[:, m, nch * 512:(nch + 1) * 512],
                           in_=kvps[m][nch][...])

    # ---------- kv2 = Wkv^T @ kv : kk rows 0:128, vv rows 128:256 ----------
    kkT = wk.tile([HID, E], F32)
    vvT = wk.tile([HID, E], F32)
    for m, dst_t in ((0, kkT), (1, vvT)):
        for nch in range(2):
            pp = pt("pa" if nch == 0 else "pb")
            for kc in range(2):
                nc.tensor.matmul(pp[...],
                                 Wkv[:, kc, m * 128:(m + 1) * 128],
                                 kvT[:, kc, nch * 512:(nch + 1) * 512],
                                 start=(kc == 0), stop=(kc == 1))
            nc.scalar.copy(out=dst_t[:, nch * 512:(nch + 1) * 512], in_=pp[...])

    # ---------- attention ----------
    dif = wk.tile([HID, E], F32)
    nc.vector.scalar_tensor_tensor(out=dif[...], in0=qTe[...], scalar=1e-6,
                                   in1=kkT[...], op0=ALU.add, op1=ALU.subtract)
    sqd = wk.tile([HID, E], BF16)
    nc.scalar.activation(out=sqd[...], in_=dif[...], func=AF.Square)
    M4 = dbc("M1", H)
    Pm = wk.tile([H, E], BF16)
    for nch in range(2):
        sl = slice(nch * 512, (nch + 1) * 512)
        pp = pt("pc", H)
        nc.tensor.matmul(pp[:H, :], hredb[...], sqd[:, sl], start=True, stop=True)
        sdt = wk.tile([H, 512], F32, tag="sdt", name="sdt")
        nc.scalar.activation(out=sdt[...], in_=pp[:H, :], func=AF.Sqrt)
        pe_ = wk.tile([H, 512], F32, tag="pe_", name="pe_")
        nc.scalar.activation(out=pe_[...], in_=sdt[...], func=AF.Exp, scale=-SCALE)
        nc.vector.tensor_tensor(out=Pm[:, sl], in0=pe_[...], in1=M4[:, sl], op=ALU.mult)
    Ssum = wk.tile([H, PCORE], F32)
    nc.vector.tensor_reduce(out=Ssum[...],
                            in_=Pm[...].rearrange("h (j k) -> h j k", k=K),
                            axis=mybir.AxisListType.X, op=ALU.add)
    Rinv = wk.tile([H, PCORE], F32)
    nc.vector.reciprocal(out=Rinv[...], in_=Ssum[...])
    Rinvb = wk.tile([H, PCORE], BF16)
    nc.vector.tensor_copy(out=Rinvb[...], in_=Rinv[...])
    ow = wk.tile([HID, PCORE], F32)
    for nch in range(2):
        sl = slice(nch * 512, (nch + 1) * 512)
        pp = pt("pd")
        nc.tensor.matmul(pp[...], hexpb[...], Pm[:, sl], start=True, stop=True)
        wv = wk.tile([HID, 512], F32, tag="wv", name="wv")
        nc.vector.tensor_tensor(out=wv[...], in0=pp[...], in1=vvT[:, sl], op=ALU.mult)
        nc.vector.tensor_reduce(out=ow[:, nch * 32:(nch + 1) * 32],
                                in_=wv[...].rearrange("c (j k) -> c j k", k=K),
                                axis=mybir.AxisListType.X, op=ALU.add)
    rfp = pt("pc")
    nc.tensor.matmul(rfp[:, :PCORE], hexpb[...], Rinvb[...], start=True, stop=True)
    oT = wk.tile([HID, PCORE], F32)
    nc.vector.tensor_tensor(out=oT[...], in0=ow[...], in1=rfp[:, :PCORE], op=ALU.mult)
    ofp = pt("pd")
    nc.tensor.matmul(ofp[:D, :PCORE], Wout[...], oT[...], start=True, stop=True)
    outFT = wk.tile([D, PCORE], F32)
    nc.scalar.copy(out=outFT[...], in_=ofp[:D, :PCORE])
    dst = bass.AP(tensor=out, offset=0, ap=[[1, D], [D, PCORE]])
    nc.sync.dma_start(out=dst, in_=outFT[...])


def _build_nc():
    nc = bacc.Bacc("TRN2", target_bir_lowering=False, debug=False,
                   num_devices=NCORES)
    P = {}
    def inp(name, shape, dt=F32):
        P[name] = nc.declare_dram_parameter(name, list(shape), dt, isOutput=False)
    inp("fT", (D, NB)); inp("nsc", (D, 1))
    inp("Wq", (D, HID), BF16); inp("Wxi", (D, D), BF16)
    inp("WxjI", (D, 2 * D), BF16)
    inp("gnb", (1, E), BF16); inp("gcb", (1, E), BF16)
    inp("rdT", (1, E), BF16); inp("M1", (1, E), BF16)
    inp("W1", (1, RH), BF16); inp("b1", (RH, 1)); inp("g1", (RH, 1))
    inp("W2", (RH, RH), BF16); inp("b2", (RH, 1)); inp("g2", (RH, 1))
    inp("W3sb", (128, 32, KVD), BF16); inp("b3T", (D, KVD), BF16)
    inp("Wkv", (128, 2, KVD), BF16); inp("Wout", (HID, D))
    inp("hredb", (128, H), BF16); inp("hexpb", (H, 128), BF16)
    inp("iota2", (128, NCH)); inp("mm64", (RH, RH), BF16)
    out = nc.declare_dram_parameter("out", [PCORE, D], F32, isOutput=True)
    import contextlib
    with tile.TileContext(nc) as tc:
        with contextlib.ExitStack() as ctx:
            _emit(nc, tc, P, out, ctx)
    nc.finalize()
    return nc


class _Runner:
    """Builds the sharded jit once; subsequent calls reuse the executable."""

    def __init__(self, nc):
        bass2jax.install_neuronx_cc_hook()
        assert nc.dbg_addr is None
        self.nc = nc
        pid = nc.partition_id_tensor
        self.partition_name = pid.name if pid else None
        in_names, out_names, out_avals = [], [], []
        for alloc in nc.m.functions[0].allocations:
            if not isinstance(alloc, mybir.MemoryLocationSet):
                continue
            name = alloc.memorylocations[0].name
            if alloc.kind == "ExternalInput":
                if name != self.partition_name:
                    in_names.append(name)
            elif alloc.kind == "ExternalOutput":
                shape = tuple(alloc.tensor_shape)
                dtype = mybir.dt.np(alloc.dtype)
                out_names.append(name)
                out_avals.append(jax.core.ShapedArray(shape, dtype))
        all_names = tuple(in_names + out_names +
                          ([self.partition_name] if self.partition_name else []))

        def _body(*args):
            operands = list(args)
            if self.partition_name is not None:
                operands.append(bass2jax.partition_id_tensor())
            return tuple(bass2jax._bass_exec_p.bind(
                *operands, out_avals=tuple(out_avals), in_names=all_names,
                out_names=tuple(out_names), lowering_input_output_aliases=(),
                sim_require_finite=True, sim_require_nnan=True, nc=nc))

        devices = jax.devices()[:NCORES]
        self.mesh = Mesh(np.asarray(devices), ("core",))
        self.sharding = NamedSharding(self.mesh, PartitionSpec("core"))
        n_in, n_out = len(in_names), len(out_names)
        self.fn = jax.jit(
            shard_map(_body, mesh=self.mesh,
                      in_specs=(PartitionSpec("core"),) * (n_in + n_out),
                      out_specs=(PartitionSpec("core"),) * n_out,
                      check_rep=False),
            donate_argnums=tuple(range(n_in, n_in + n_out)),
            keep_unused=True)
        self.in_names = in_names
        self.out_avals = out_avals

    def put(self, a):
        return jax.device_put(np.ascontiguousarray(a), self.sharding)

    def rep(self, a):
        """Replicate a per-core operand into the global (8*rows, ...) layout."""
        a = np.ascontiguousarray(a)
        g = np.broadcast_to(a[None], (NCORES,) + a.shape)
        return self.put(g.reshape(NCORES * a.shape[0], *a.shape[1:]))

    def dispatch(self, by_name):
        args = [by_name[n] for n in self.in_names]
        zeros = [np.zeros((NCORES * av.shape[0], *av.shape[1:]), av.dtype)
                 for av in self.out_avals]
        return self.fn(*args, *zeros)

    def run(self, by_name):
        return np.asarray(self.dispatch(by_name)[0])


_STATE = None


def _init_state():
    bf = ml_dtypes.bfloat16
    nc = _build_nc()
    r = _Runner(nc)

    hred = np.zeros((128, H), bf)
    for h in range(H):
        hred[h * 32:(h + 1) * 32, h] = 1
    hexp = np.ascontiguousarray(hred.T)
    iota2 = (np.arange(128, dtype=np.float32)[:, None]
             + 128.0 * np.arange(NCH, dtype=np.float32)[None, :])
    # center index row differs per core: within-batch node id repeated K times
    gcb = np.empty((NCORES, E), bf)
    for c in range(NCORES):
        loc = (c % (NCORES // B)) * PCORE + np.arange(PCORE)
        gcb[c] = np.repeat(loc, K).astype(bf)

    const = dict(
        hredb=r.rep(hred), hexpb=r.rep(hexp),
        iota2=r.rep(iota2), gcb=r.put(gcb),
        mm64=r.rep(np.full((RH, RH), 1.0 / RH, bf)),
    )
    return {"runner": r, "const": const, "wkey": None, "wdev": None}


def _weights_key(ws):
    h = hashlib.blake2b(digest_size=16)
    for a in ws:
        a = np.asarray(a)
        h.update(str(a.shape).encode()); h.update(str(a.dtype).encode())
        h.update(np.ascontiguousarray(a).tobytes())
    return h.digest()


def _prep_weights(r, norm_scale, Wq, Wxi, Wxj, rp_W1, rp_b1, rp_g1,
                  rp_W2, rp_b2, rp_g2, rp_W3, rp_b3, Wkv_out, Wout):
    bf = ml_dtypes.bfloat16
    WxjI = np.concatenate([np.asarray(Wxj, np.float32),
                           np.eye(D, dtype=np.float32)], axis=1)
    W3sb = np.ascontiguousarray(
        np.asarray(rp_W3, np.float32)
        .reshape(RH, KVD, D).transpose(0, 2, 1)       # (r, d, o)
        .reshape(RH * D, KVD)                         # row = r*64 + d
        .reshape(32, 128, KVD).transpose(1, 0, 2)     # (p, chunk, o)
    ).astype(bf)
    b3T = np.ascontiguousarray(
        np.asarray(rp_b3, np.float32).reshape(KVD, D).T).astype(bf)
    WkvP = np.ascontiguousarray(
        np.asarray(Wkv_out, np.float32).reshape(2, 128, KVD).transpose(1, 0, 2))
    return dict(
        nsc=r.rep(np.asarray(norm_scale, np.float32).reshape(D, 1)),
        Wq=r.rep(np.asarray(Wq, np.float32).astype(bf)),
        Wxi=r.rep(np.asarray(Wxi, np.float32).astype(bf)),
        WxjI=r.rep(WxjI.astype(bf)),
        W1=r.rep(np.asarray(rp_W1, np.float32).reshape(1, RH).astype(bf)),
        b1=r.rep(np.asarray(rp_b1, np.float32).reshape(RH, 1)),
        g1=r.rep(np.asarray(rp_g1, np.float32).reshape(RH, 1)),
        W2=r.rep(np.asarray(rp_W2, np.float32).astype(bf)),
        b2=r.rep(np.asarray(rp_b2, np.float32).reshape(RH, 1)),
        g2=r.rep(np.asarray(rp_g2, np.float32).reshape(RH, 1)),
        W3sb=r.rep(W3sb), b3T=r.rep(b3T), Wkv=r.rep(WkvP.astype(bf)),
        Wout=r.rep(np.asarray(Wout, np.float32)),
    )


def kernel(features, neighbor_indices, neighbor_mask, rel_dist, norm_scale,
           Wq, Wxi, Wxj, rp_W1, rp_b1, rp_g1, rp_W2, rp_b2, rp_g2,
           rp_W3, rp_b3, Wkv_out, Wout):
    global _STATE
    if _STATE is None:
        _STATE = _init_state()
    st = _STATE
    r = st["runner"]

    wlist = (norm_scale, Wq, Wxi, Wxj, rp_W1, rp_b1, rp_g1, rp_W2, rp_b2,
             rp_g2, rp_W3, rp_b3, Wkv_out, Wout)

    f = np.asarray(features, np.float32)
    idx = np.asarray(neighbor_indices)
    msk = np.asarray(neighbor_mask)
    rd = np.asarray(rel_dist, np.float32)

    # per-core activations, laid out as the global (8*rows, ...) arrays
    fb = np.ascontiguousarray(f[..., 0].transpose(0, 2, 1))       # (B, D, N)
    fT = np.broadcast_to(fb[:, None], (B, NCORES // B, D, NB)) \
           .reshape(NCORES * D, NB)                               # core c -> batch c//4
    bf = ml_dtypes.bfloat16
    acts = dict(fT=np.ascontiguousarray(fT),
                gnb=idx.reshape(NCORES, E).astype(bf),
                rdT=rd.reshape(NCORES, E).astype(bf),
                M1=msk.reshape(NCORES, E).astype(bf))

    if st["wkey"] is not None:
        # optimistic: launch with the cached weights, hash while it flies
        outs = r.dispatch({**st["const"], **st["wdev"], **acts})
        wkey = _weights_key(wlist)
        if wkey == st["wkey"]:
            out = np.asarray(outs[0])
            return out.reshape(B, N, D, 1).astype(np.float32)
        del outs  # weights changed: discard the speculative launch
    else:
        wkey = _weights_key(wlist)
    st["wdev"] = _prep_weights(r, *wlist)
    st["wkey"] = wkey
    out = r.run({**st["const"], **st["wdev"], **acts})            # (8*PCORE, D)
    return out.reshape(B, N, D, 1).astype(np.float32)
